# revision 4
# baseline (speedup 1.0000x reference)
"""Trainium2 Bass kernel v2 for nn_CudaFastWeightSumPerformerLayer.

Instruction-count-minimized rewrite of the chunked Performer FAVOR+
causal linear attention layer. Sharding: 8 cores = 4 batches x 2
head-groups (4 heads each); host sums the two partials per batch and
applies residual + LayerNorm.

Key structure (vs v1):
  - all-bf16 pipeline (f32 PSUM), fp16 output, bf16 packed inputs
    (2 input tensors instead of 6)
  - scan chunk 512 with transposed outputs out^T[d, t]: inter/intra/
    state-update are up-to-512-col matmuls; per-column normalization
    via Pool partition_broadcast of the reciprocal denominator row
  - state S^T accumulates in a PSUM bank across all chunks (no DVE
    adds); per chunk it is snapshotted + PE-transposed to give the
    feature-major state for the next chunk's inter matmul
  - token-major k features for head pairs via one matmul per token
    block (block-diagonal proj stacking)
  - ksum via one segmented DVE reduce per head; 1/ksum folded into
    the token-major V tile (columns [v*rk | rk | 0])
out_final = out_raw / (denom_raw + eps * qsum); residual + LN on host.
"""

import numpy as np

L, DM, DH, M = 2048, 512, 64, 256
F = 2 * M
HPC = 4            # heads per core
B = 4
CH = 512           # scan chunk
NCH = L // CH      # 4
NJB = CH // 128    # 4 j-blocks per chunk
SCALE = DH ** -0.5
EPS_ATTN = 1e-5
EPS_LN = 1e-5
N_CORES = 8

_CACHE = {}


def _build_nc():
    import concourse.bacc as bacc
    import concourse.tile as tile
    from concourse import mybir

    f32 = mybir.dt.float32
    bf16 = mybir.dt.bfloat16
    fp16 = mybir.dt.float16
    AF = mybir.ActivationFunctionType
    ALU = mybir.AluOpType
    AX = mybir.AxisListType

    nc = bacc.Bacc("TRN2", target_bir_lowering=False, debug=False,
                   num_devices=N_CORES)

    hTb_d = nc.dram_tensor("hTb", [DM, L], bf16, kind="ExternalInput")
    # wpack columns: 0:512 wqkT ([q0q1|k0k1|q2q3|k2k3] col-blocks of 128),
    # 512:768 wvT, 768:1280 woT (rows 0:256), 1536:2048 aux block:
    #   rows 0:128   = bdproj (block-diag proj for head pairs)
    #   rows 128:256 = [proj2 (256) | ident (128) | unused]
    #   rows 256:384 = mask_const [tri(128) | ones(384)]
    wpack_d = nc.dram_tensor("wpack", [DM, 2048], bf16, kind="ExternalInput")
    part_d = nc.dram_tensor("part", [L, DM], fp16, kind="ExternalOutput")

    with tile.TileContext(nc) as tc:
        from contextlib import ExitStack
        with ExitStack() as ctx:
            consts = ctx.enter_context(tc.tile_pool(name="consts", bufs=1))
            qkp = ctx.enter_context(tc.tile_pool(name="qkp", bufs=1))
            vnp = ctx.enter_context(tc.tile_pool(name="vnp", bufs=1))
            onp = ctx.enter_context(tc.tile_pool(name="onp", bufs=1))
            rkp = ctx.enter_context(tc.tile_pool(name="rkp", bufs=1))

            wqk_sb = consts.tile([128, 4, 512], bf16, tag="wqk", name="wqk")
            wv_sb = consts.tile([128, 4, 256], bf16, tag="wv", name="wv")
            wo_sb = consts.tile([128, 2, 512], bf16, tag="wo", name="wo")
            bdproj = consts.tile([128, 512], bf16, tag="bdp", name="bdp")
            pim = consts.tile([128, 512], bf16, tag="pim", name="pim")
            mask_sb = consts.tile([128, 512], bf16, tag="mask", name="mask")
            epsc = consts.tile([128, 1], f32, tag="epsc", name="epsc")
            nc.vector.memset(epsc, EPS_ATTN)
            onec = consts.tile([128, 1], bf16, tag="onec", name="onec")
            nc.vector.memset(onec, 1.0)
            nc.sync.dma_start(
                out=wqk_sb,
                in_=wpack_d[:, 0:512].rearrange("(k p) n -> p k n", p=128))
            nc.sync.dma_start(
                out=wv_sb,
                in_=wpack_d[:, 512:768].rearrange("(k p) n -> p k n", p=128))
            nc.sync.dma_start(
                out=wo_sb,
                in_=wpack_d[0:256, 768:1280].rearrange(
                    "(k p) n -> p k n", p=128))
            nc.sync.dma_start(out=bdproj, in_=wpack_d[0:128, 1536:2048])
            nc.sync.dma_start(out=pim, in_=wpack_d[128:256, 1536:2048])
            nc.sync.dma_start(out=mask_sb, in_=wpack_d[256:384, 1536:2048])
            proj2 = pim[:, 0:256]
            ident = pim[:, 256:384]

            qq_sb = [qkp.tile([128, L], bf16, tag=f"qq{g}", name=f"qq{g}")
                     for g in range(2)]
            kk_sb = [qkp.tile([128, L], bf16, tag=f"kk{g}", name=f"kk{g}")
                     for g in range(2)]
            # vn: [t128, block, head, 66] = [v*rk | rk | 0]
            vn_sb = vnp.tile([128, 16, 4, 66], bf16, tag="vn", name="vn")
            nc.vector.memset(vn_sb[:, :, :, 64:66], 0.0)
            rk_sb = rkp.tile([128, 16, 4], f32, tag="rk", name="rk")
            ks_sb = rkp.tile([128, 16, 4], f32, tag="ks", name="ks")
            # normalized outputs, hd-major: [hd 128 (2 heads), pb, t]
            onT_sb = onp.tile([128, 2, L], bf16, tag="onT", name="onT")

            # ---- phase 1: qkv projection ----
            with tc.tile_pool(name="hTp", bufs=1) as hTp, \
                 tc.tile_pool(name="p1ps", bufs=1, space="PSUM") as p1ps, \
                 tc.tile_pool(name="p1vps", bufs=4, space="PSUM") as p1vps:
                hT_all = hTp.tile([128, 4, L], bf16, tag="hTa", name="hTa")
                for kc in range(4):
                    nc.sync.dma_start(
                        out=hT_all[:, kc, :],
                        in_=hTb_d[128 * kc:128 * (kc + 1), :])
                dst = [qq_sb[0], kk_sb[0], qq_sb[1], kk_sb[1]]
                for cb in range(4):
                    ps = p1ps.tile([128, L], f32, tag="qkps", name="qkps")
                    for kc in range(4):
                        for t4 in range(4):
                            nc.tensor.matmul(
                                ps[:, 512 * t4:512 * (t4 + 1)],
                                wqk_sb[:, kc, 128 * cb:128 * (cb + 1)],
                                hT_all[:, kc, 512 * t4:512 * (t4 + 1)],
                                start=(kc == 0), stop=(kc == 3))
                    nc.scalar.copy(out=dst[cb][:], in_=ps[:])
                for tp in range(8):
                    ps = p1vps.tile([128, 512], f32, tag="vps", name="vps")
                    for sub in range(2):
                        tb = 2 * tp + sub
                        for kc in range(4):
                            nc.tensor.matmul(
                                ps[:, 256 * sub:256 * (sub + 1)],
                                hT_all[:, kc, 128 * tb:128 * (tb + 1)],
                                wv_sb[:, kc, :],
                                start=(kc == 0), stop=(kc == 3))
                    nc.scalar.copy(
                        out=vn_sb[:, 2 * tp:2 * tp + 2, :, 0:64],
                        in_=ps[:].rearrange("p (a b c) -> p a b c", b=4,
                                            c=64))

            # ---- phases 2+3: features + scan, pipelined over heads ----
            with tc.tile_pool(name="feat", bufs=1) as featp, \
                 tc.tile_pool(name="stsb", bufs=1) as stp, \
                 tc.tile_pool(name="bmp", bufs=1) as bmp, \
                 tc.tile_pool(name="rowp", bufs=2) as rowp, \
                 tc.tile_pool(name="dps", bufs=1, space="PSUM") as dps, \
                 tc.tile_pool(name="bkps", bufs=2, space="PSUM") as bkps, \
                 tc.tile_pool(name="ops", bufs=2, space="PSUM") as ops, \
                 tc.tile_pool(name="stps", bufs=1, space="PSUM") as stps, \
                 tc.tile_pool(name="trp", bufs=1, space="PSUM") as trp:

                def features(m, out):
                    """Generator: emits feature ops in slabs, yielding
                    between slabs so scan chunks can interleave."""
                    p = m % 2
                    g = m // 2
                    half = slice(64 * p, 64 * (p + 1))
                    qp = featp.tile([128, 4, L], bf16, tag=f"qp{p}",
                                    name=f"qp{p}")
                    kp = featp.tile([128, 4, L], bf16, tag=f"kp{p}",
                                    name=f"kp{p}")
                    kt = None
                    if p == 0:
                        kt = featp.tile([128, 16, 2, 512], bf16,
                                        tag=f"kt{g}", name=f"kt{g}")
                    out[m] = (qp, kp, kt)
                    if p == 0:
                        # token-major k features first: the fold_pair
                        # chain (act->DVE->Pool) then overlaps this
                        # head's own feature-major matmuls
                        for tb in range(16):
                            dt_ps = dps.tile([128, 512], f32, tag="dps",
                                             name="dps")
                            nc.tensor.matmul(
                                dt_ps[:],
                                kk_sb[g][:, 128 * tb:128 * (tb + 1)],
                                bdproj[:],
                                start=True, stop=True)
                            nc.scalar.activation(
                                out=kt[:, tb, :, 0:256],
                                in_=dt_ps[:].rearrange("p (a b) -> p a b",
                                                       a=2),
                                func=AF.Exp)
                            if tb % 4 == 3:
                                yield
                        with nc.allow_low_precision(reason="bf16 features"):
                            nc.vector.reciprocal(out=kt[:, :, :, 256:512],
                                                 in_=kt[:, :, :, 0:256])
                        fold_pair(g, kt)
                        yield
                    for (src, dstf) in ((qq_sb[g], qp), (kk_sb[g], kp)):
                        for fh in range(2):
                            for t4 in range(4):
                                d_ps = dps.tile([128, 512], f32, tag="dps",
                                                name="dps")
                                nc.tensor.matmul(
                                    d_ps[:],
                                    proj2[half, 128 * fh:128 * (fh + 1)],
                                    src[half, 512 * t4:512 * (t4 + 1)],
                                    start=True, stop=True)
                                nc.scalar.activation(
                                    out=dstf[:, fh,
                                             512 * t4:512 * (t4 + 1)],
                                    in_=d_ps[:], func=AF.Exp)
                                if t4 % 2 == 1:
                                    yield
                            with nc.allow_low_precision(
                                    reason="bf16 features"):
                                nc.vector.reciprocal(
                                    out=dstf[:, 2 + fh, :],
                                    in_=dstf[:, fh, :])
                            yield

                def fold_pair(g, kt):
                    for p in range(2):
                        m = 2 * g + p
                        nc.vector.tensor_reduce(
                            out=ks_sb[:, :, m], in_=kt[:, :, p, :],
                            axis=AX.X, op=ALU.add)
                        nc.vector.reciprocal(out=rk_sb[:, :, m],
                                             in_=ks_sb[:, :, m])
                        with nc.allow_low_precision(reason="bf16 vn"):
                            nc.gpsimd.tensor_tensor(
                                out=vn_sb[:, :, m, 0:64],
                                in0=vn_sb[:, :, m, 0:64],
                                in1=rk_sb[:, :, m].unsqueeze(-1)
                                .to_broadcast([128, 16, 64]),
                                op=ALU.mult)
                            nc.gpsimd.tensor_copy(
                                out=vn_sb[:, :, m, 64:65],
                                in_=rk_sb[:, :, m].unsqueeze(-1))

                def scan(m, qp, kp, kt):
                    p = m % 2
                    st = stp.tile([128, 4, 66], bf16, tag=f"st{p}",
                                  name=f"st{p}")
                    nc.vector.memset(st, 0.0)
                    nc.vector.memset(st[:, :, 64:65], EPS_ATTN)
                    stT_ps = stps.tile([66, 512], f32, tag=f"sT{p}",
                                       name=f"sT{p}")
                    stT_sb = stp.tile([66, 512], bf16, tag=f"sTs{p}",
                                      name=f"sTs{p}")
                    for c in range(NCH):
                        tsl = slice(CH * c, CH * (c + 1))
                        o_ps = ops.tile([66, CH], f32, tag="o", name="o")
                        bms = []
                        for jb in range(NJB):
                            jbg = NJB * c + jb
                            cols = CH - 128 * jb
                            jsl = slice(CH * c + 128 * jb, CH * (c + 1))
                            bk = bkps.tile([128, 512], f32, tag="bk",
                                           name="bk")
                            for fc in range(4):
                                nc.tensor.matmul(
                                    bk[:, 0:cols],
                                    kp[:, fc, 128 * jbg:128 * (jbg + 1)],
                                    qp[:, fc, jsl],
                                    start=(fc == 0), stop=(fc == 3))
                            bm = bmp.tile([128, 512], bf16,
                                          tag=f"bm{jb}", name=f"bm{jb}")
                            with nc.allow_low_precision(reason="bf16 B"):
                                nc.vector.tensor_mul(
                                    out=bm[:, 0:cols], in0=bk[:, 0:cols],
                                    in1=mask_sb[:, 0:cols])
                            bms.append(bm)
                        # out^T: intra (starts the psum group) then inter
                        for jb in range(NJB):
                            jbg = NJB * c + jb
                            cols = CH - 128 * jb
                            nc.tensor.matmul(
                                o_ps[:, 128 * jb:CH],
                                vn_sb[:, jbg, m, :], bms[jb][:, 0:cols],
                                start=(jb == 0), stop=False,
                                skip_group_check=True)
                            nc.tensor.matmul(
                                stT_ps[:],
                                vn_sb[:, jbg, m, :], kt[:, jbg, p, :],
                                start=(c == 0 and jb == 0),
                                stop=(c == NCH - 1 and jb == NJB - 1),
                                skip_group_check=True)
                        for fc in range(4):
                            nc.tensor.matmul(
                                o_ps[:], st[:, fc, :], qp[:, fc, tsl],
                                start=False, stop=(fc == 3),
                                skip_group_check=True)
                        if c < NCH - 1:
                            nc.scalar.copy(out=stT_sb[:], in_=stT_ps[:])
                            for fc in range(4):
                                t_ps = trp.tile([128, 66], bf16, tag="tr",
                                                 name="tr")
                                nc.tensor.transpose(
                                    t_ps[:],
                                    stT_sb[:, 128 * fc:128 * (fc + 1)],
                                    ident[0:66, 0:66])
                                nc.scalar.copy(out=st[:, fc, 0:64],
                                               in_=t_ps[:, 0:64])
                                nc.scalar.add(out=st[:, fc, 64:65],
                                              in_=t_ps[:, 64:65],
                                              add=epsc[:, 0:1])
                        nr = rowp.tile([1, CH], f32, tag=f"nr{p}",
                                       name=f"nr{p}")
                        nb = rowp.tile([128, CH], f32, tag=f"nb{p}",
                                       name=f"nb{p}")
                        nc.vector.reciprocal(out=nr[:], in_=o_ps[64:65, :])
                        nc.gpsimd.partition_broadcast(out_ap=nb[:],
                                                      in_ap=nr[:])
                        with nc.allow_low_precision(reason="bf16 out"):
                            nc.vector.tensor_mul(
                                out=onT_sb[64 * p:64 * (p + 1), m // 2,
                                           tsl],
                                in0=o_ps[0:64, :], in1=nb[0:64, :])
                        yield

                def drain(gen):
                    for _ in gen:
                        pass

                # software pipeline: interleave features(m+1) slabs with
                # scan(m) chunks so PE never stalls on activation drains
                feats = {}
                drain(features(0, feats))
                for m in range(1, 4):
                    f_gen = features(m, feats)
                    mm = m - 1
                    s_gen = scan(mm, feats[mm][0], feats[mm][1],
                                 feats[2 * (mm // 2)][2])
                    alive = True
                    while alive:
                        alive = False
                        for _ in range(3):
                            if next(f_gen, 'end') != 'end':
                                alive = True
                        if next(s_gen, 'end') != 'end':
                            alive = True
                    drain(f_gen)
                    drain(s_gen)
                drain(scan(3, feats[3][0], feats[3][1], feats[2][2]))

            # ---- phase 5: partial attn = out_norm @ woT (fp16) ----
            with tc.tile_pool(name="atsb", bufs=2) as atsbp, \
                 tc.tile_pool(name="atps", bufs=2, space="PSUM") as atps:
                part_v = part_d[:].rearrange("(g p) d -> p g d", p=128)
                for g in range(4):
                    a_sb = atsbp.tile([128, 4, DM], fp16, tag="atsb",
                                      name="atsb")
                    for cc in range(4):
                        tb = 4 * g + cc
                        a_ps = atps.tile([128, DM], f32, tag="atps",
                                         name="atps")
                        for pb in range(2):
                            nc.tensor.matmul(
                                a_ps[:],
                                onT_sb[:, pb, 128 * tb:128 * (tb + 1)],
                                wo_sb[:, pb, :],
                                start=(pb == 0), stop=(pb == 1))
                        nc.scalar.copy(out=a_sb[:, cc, :], in_=a_ps[:])
                    nc.sync.dma_start(out=part_v[:, 4 * g:4 * (g + 1), :],
                                      in_=a_sb[:])

    nc.compile()
    return nc


def _host_prep(h, w_qkv, w_o, proj_matrix):
    """Build per-core packed bf16 input maps."""
    import ml_dtypes
    bf = ml_dtypes.bfloat16
    projs = (proj_matrix * (DH ** -0.25)).astype(np.float32)  # (64, 256)
    proj2 = np.concatenate([projs, projs], axis=0)            # (128, 256)
    bdproj = np.zeros((128, 512), np.float32)
    bdproj[0:64, 0:256] = projs
    bdproj[64:128, 256:512] = projs
    ident = np.eye(128, dtype=np.float32)
    tri = (np.arange(128)[:, None] <= np.arange(128)[None, :]
           ).astype(np.float32)
    mask_const = np.concatenate([tri, np.ones((128, 384), np.float32)],
                                axis=1)
    woT_full = (w_o.T * SCALE).astype(np.float32)  # (512, 512)

    in_maps = []
    for core in range(N_CORES):
        b, hg = core // 2, core % 2
        heads = [HPC * hg + mm for mm in range(HPC)]
        hT = np.ascontiguousarray(h[:, b, :].T)
        wpack = np.zeros((DM, 2048), np.float32)
        for g in range(2):
            for p in range(2):
                hh = heads[2 * g + p]
                blk = w_qkv[192 * hh:192 * (hh + 1)]  # (192, DM) [q,k,v]
                c0 = 256 * g + 64 * p
                wpack[:, c0:c0 + 64] = blk[0:64].T
                wpack[:, c0 + 128:c0 + 192] = blk[64:128].T
                i = 2 * g + p
                wpack[:, 512 + 64 * i:512 + 64 * (i + 1)] = blk[128:192].T
        for i, hh in enumerate(heads):
            wpack[64 * i:64 * (i + 1), 768:1280] = \
                woT_full[64 * hh:64 * (hh + 1), :]
        wpack[0:128, 1536:2048] = bdproj
        wpack[128:256, 1536:1792] = proj2
        wpack[128:256, 1792:1920] = ident
        wpack[256:384, 1536:2048] = mask_const
        in_maps.append({
            "hTb": hT.astype(bf),
            "wpack": wpack.astype(bf),
        })
    return in_maps


def kernel(h, w_qkv, w_o, ln_gamma, ln_beta, proj_matrix):
    from concourse.bass_utils import run_bass_kernel_spmd

    h = np.asarray(h, np.float32)
    w_qkv = np.asarray(w_qkv, np.float32)
    w_o = np.asarray(w_o, np.float32)
    ln_gamma = np.asarray(ln_gamma, np.float32)
    ln_beta = np.asarray(ln_beta, np.float32)
    proj_matrix = np.asarray(proj_matrix, np.float32)

    if "nc" not in _CACHE:
        _CACHE["nc"] = _build_nc()
    nc = _CACHE["nc"]

    in_maps = _host_prep(h, w_qkv, w_o, proj_matrix)
    res = run_bass_kernel_spmd(nc, in_maps, core_ids=list(range(N_CORES)))

    out = np.empty((L, B, DM), np.float32)
    for b in range(B):
        attn = (np.asarray(res.results[2 * b]["part"], np.float32)
                + np.asarray(res.results[2 * b + 1]["part"], np.float32))
        x = h[:, b, :] + attn
        mu = x.mean(-1, keepdims=True)
        var = ((x - mu) ** 2).mean(-1, keepdims=True)
        out[:, b, :] = (x - mu) / np.sqrt(var + EPS_LN) * ln_gamma + ln_beta
    return out


# revision 5
# speedup vs baseline: 1.2980x; 1.2980x over previous
"""Trainium2 Bass kernel v2 for nn_CudaFastWeightSumPerformerLayer.

Instruction-count-minimized rewrite of the chunked Performer FAVOR+
causal linear attention layer. Sharding: 8 cores = 4 batches x 2
head-groups (4 heads each); host sums the two partials per batch and
applies residual + LayerNorm.

Key structure (vs v1):
  - all-bf16 pipeline (f32 PSUM), fp16 output, bf16 packed inputs
    (2 input tensors instead of 6)
  - scan chunk 512 with transposed outputs out^T[d, t]: inter/intra/
    state-update are up-to-512-col matmuls; per-column normalization
    via Pool partition_broadcast of the reciprocal denominator row
  - state S^T accumulates in a PSUM bank across all chunks (no DVE
    adds); per chunk it is snapshotted + PE-transposed to give the
    feature-major state for the next chunk's inter matmul
  - token-major k features for head pairs via one matmul per token
    block (block-diagonal proj stacking)
  - ksum via one segmented DVE reduce per head; 1/ksum folded into
    the token-major V tile (columns [v*rk | rk | 0])
out_final = out_raw / (denom_raw + eps * qsum); residual + LN on host.
"""

import numpy as np

L, DM, DH, M = 2048, 512, 64, 256
F = 2 * M
HPC = 4            # heads per core
B = 4
CH = 512           # scan chunk
NCH = L // CH      # 4
NJB = CH // 128    # 4 j-blocks per chunk
SCALE = DH ** -0.5
EPS_ATTN = 1e-5
EPS_LN = 1e-5
N_CORES = 8

_CACHE = {}


def _build_nc():
    import concourse.bacc as bacc
    import concourse.tile as tile
    from concourse import mybir

    f32 = mybir.dt.float32
    bf16 = mybir.dt.bfloat16
    fp16 = mybir.dt.float16
    AF = mybir.ActivationFunctionType
    ALU = mybir.AluOpType
    AX = mybir.AxisListType

    nc = bacc.Bacc("TRN2", target_bir_lowering=False, debug=False,
                   num_devices=N_CORES)

    hTb_d = nc.dram_tensor("hTb", [DM, L], bf16, kind="ExternalInput")
    # wpack columns: 0:512 wqkT ([q0q1|k0k1|q2q3|k2k3] col-blocks of 128),
    # 512:768 wvT, 768:1280 woT (rows 0:256), 1536:2048 aux block:
    #   rows 0:128   = bdproj (block-diag proj for head pairs)
    #   rows 128:256 = [proj2 (256) | ident (128) | unused]
    #   rows 256:384 = mask_const [tri(128) | ones(384)]
    wpack_d = nc.dram_tensor("wpack", [DM, 2048], bf16, kind="ExternalInput")
    part_d = nc.dram_tensor("part", [L, DM], fp16, kind="ExternalOutput")

    with tile.TileContext(nc) as tc:
        from contextlib import ExitStack
        with ExitStack() as ctx:
            consts = ctx.enter_context(tc.tile_pool(name="consts", bufs=1))
            qkp = ctx.enter_context(tc.tile_pool(name="qkp", bufs=1))
            vnp = ctx.enter_context(tc.tile_pool(name="vnp", bufs=1))
            onp = ctx.enter_context(tc.tile_pool(name="onp", bufs=1))
            rkp = ctx.enter_context(tc.tile_pool(name="rkp", bufs=1))

            wqk_sb = consts.tile([128, 4, 512], bf16, tag="wqk", name="wqk")
            wv_sb = consts.tile([128, 4, 256], bf16, tag="wv", name="wv")
            wo_sb = consts.tile([128, 2, 512], bf16, tag="wo", name="wo")
            bdproj = consts.tile([128, 512], bf16, tag="bdp", name="bdp")
            pim = consts.tile([128, 512], bf16, tag="pim", name="pim")
            mask_sb = consts.tile([128, 512], bf16, tag="mask", name="mask")
            epsc = consts.tile([128, 1], f32, tag="epsc", name="epsc")
            nc.vector.memset(epsc, EPS_ATTN)
            nc.sync.dma_start(
                out=wqk_sb,
                in_=wpack_d[:, 0:512].rearrange("(k p) n -> p k n", p=128))
            nc.sync.dma_start(out=pim, in_=wpack_d[128:256, 1536:2048])
            proj2 = pim[:, 0:256]
            ident = pim[:, 256:384]

            qq_sb = [qkp.tile([128, L], bf16, tag=f"qq{g}", name=f"qq{g}")
                     for g in range(2)]
            kk_sb = [qkp.tile([128, L], bf16, tag=f"kk{g}", name=f"kk{g}")
                     for g in range(2)]
            # vn: [t128, block, head, 66] = [v*rk | rk | 0]
            vn_sb = vnp.tile([128, 16, 4, 66], bf16, tag="vn", name="vn")
            nc.vector.memset(vn_sb[:, :, :, 64:66], 0.0)
            rk_sb = rkp.tile([128, 16, 4], f32, tag="rk", name="rk")
            ks_sb = rkp.tile([128, 16, 4], f32, tag="ks", name="ks")
            # normalized outputs, hd-major: [hd 128 (2 heads), pb, t]
            onT_sb = onp.tile([128, 2, L], bf16, tag="onT", name="onT")

            # ---- phase 1: qkv projection ----
            with tc.tile_pool(name="hTp", bufs=1) as hTp, \
                 tc.tile_pool(name="p1ps", bufs=1, space="PSUM") as p1ps, \
                 tc.tile_pool(name="p1vps", bufs=4, space="PSUM") as p1vps:
                hT_all = hTp.tile([128, 4, L], bf16, tag="hTa", name="hTa")
                for kc in range(4):
                    nc.sync.dma_start(
                        out=hT_all[:, kc, :],
                        in_=hTb_d[128 * kc:128 * (kc + 1), :])
                nc.sync.dma_start(
                    out=wv_sb,
                    in_=wpack_d[:, 512:768].rearrange("(k p) n -> p k n",
                                                      p=128))
                nc.sync.dma_start(out=bdproj,
                                  in_=wpack_d[0:128, 1536:2048])
                nc.sync.dma_start(out=mask_sb,
                                  in_=wpack_d[256:384, 1536:2048])
                nc.sync.dma_start(
                    out=wo_sb,
                    in_=wpack_d[0:256, 768:1280].rearrange(
                        "(k p) n -> p k n", p=128))
                dst = [qq_sb[0], kk_sb[0], qq_sb[1], kk_sb[1]]
                for cb in range(4):
                    ps = p1ps.tile([128, L], f32, tag="qkps", name="qkps")
                    for kc in range(4):
                        for t4 in range(4):
                            nc.tensor.matmul(
                                ps[:, 512 * t4:512 * (t4 + 1)],
                                wqk_sb[:, kc, 128 * cb:128 * (cb + 1)],
                                hT_all[:, kc, 512 * t4:512 * (t4 + 1)],
                                start=(kc == 0), stop=(kc == 3))
                    nc.scalar.copy(out=dst[cb][:], in_=ps[:])
                for tp in range(8):
                    ps = p1vps.tile([128, 512], f32, tag="vps", name="vps")
                    for sub in range(2):
                        tb = 2 * tp + sub
                        for kc in range(4):
                            nc.tensor.matmul(
                                ps[:, 256 * sub:256 * (sub + 1)],
                                hT_all[:, kc, 128 * tb:128 * (tb + 1)],
                                wv_sb[:, kc, :],
                                start=(kc == 0), stop=(kc == 3))
                    nc.scalar.copy(
                        out=vn_sb[:, 2 * tp:2 * tp + 2, :, 0:64],
                        in_=ps[:].rearrange("p (a b c) -> p a b c", b=4,
                                            c=64))

            # ---- phases 2+3: features + scan, pipelined over heads ----
            with tc.tile_pool(name="feat", bufs=1) as featp, \
                 tc.tile_pool(name="stsb", bufs=1) as stp, \
                 tc.tile_pool(name="bmp", bufs=1) as bmp, \
                 tc.tile_pool(name="rowp", bufs=2) as rowp, \
                 tc.tile_pool(name="dps", bufs=1, space="PSUM") as dps, \
                 tc.tile_pool(name="bkps", bufs=2, space="PSUM") as bkps, \
                 tc.tile_pool(name="ops", bufs=2, space="PSUM") as ops, \
                 tc.tile_pool(name="stps", bufs=1, space="PSUM") as stps, \
                 tc.tile_pool(name="trp", bufs=1, space="PSUM") as trp:

                def features(m, out):
                    """Generator: emits feature ops in slabs, yielding
                    between slabs so scan chunks can interleave."""
                    p = m % 2
                    g = m // 2
                    half = slice(64 * p, 64 * (p + 1))
                    qp = featp.tile([128, 4, L], bf16, tag=f"qp{p}",
                                    name=f"qp{p}")
                    kp = featp.tile([128, 4, L], bf16, tag=f"kp{p}",
                                    name=f"kp{p}")
                    kt = None
                    if p == 0:
                        kt = featp.tile([128, 16, 2, 512], bf16,
                                        tag=f"kt{g}", name=f"kt{g}")
                    out[m] = (qp, kp, kt)
                    if p == 0:
                        # token-major k features first: the fold_pair
                        # chain (act->DVE->Pool) then overlaps this
                        # head's own feature-major matmuls
                        for tb in range(16):
                            dt_ps = dps.tile([128, 512], f32, tag="dps",
                                             name="dps")
                            nc.tensor.matmul(
                                dt_ps[:],
                                kk_sb[g][:, 128 * tb:128 * (tb + 1)],
                                bdproj[:],
                                start=True, stop=True)
                            nc.scalar.activation(
                                out=kt[:, tb, :, 0:256],
                                in_=dt_ps[:].rearrange("p (a b) -> p a b",
                                                       a=2),
                                func=AF.Exp)
                            if tb % 4 == 3:
                                yield
                        with nc.allow_low_precision(reason="bf16 features"):
                            nc.vector.reciprocal(out=kt[:, :, :, 256:512],
                                                 in_=kt[:, :, :, 0:256])
                        fold_pair(g, kt)
                        yield
                    for (src, dstf) in ((qq_sb[g], qp), (kk_sb[g], kp)):
                        for fh in range(2):
                            for t4 in range(4):
                                d_ps = dps.tile([128, 512], f32, tag="dps",
                                                name="dps")
                                nc.tensor.matmul(
                                    d_ps[:],
                                    proj2[half, 128 * fh:128 * (fh + 1)],
                                    src[half, 512 * t4:512 * (t4 + 1)],
                                    start=True, stop=True)
                                nc.scalar.activation(
                                    out=dstf[:, fh,
                                             512 * t4:512 * (t4 + 1)],
                                    in_=d_ps[:], func=AF.Exp)
                                if t4 % 2 == 1:
                                    yield
                            with nc.allow_low_precision(
                                    reason="bf16 features"):
                                nc.vector.reciprocal(
                                    out=dstf[:, 2 + fh, :],
                                    in_=dstf[:, fh, :])
                            yield

                def fold_pair(g, kt):
                    for p in range(2):
                        m = 2 * g + p
                        nc.vector.tensor_reduce(
                            out=ks_sb[:, :, m], in_=kt[:, :, p, :],
                            axis=AX.X, op=ALU.add)
                        nc.vector.reciprocal(out=rk_sb[:, :, m],
                                             in_=ks_sb[:, :, m])
                        with nc.allow_low_precision(reason="bf16 vn"):
                            nc.gpsimd.tensor_tensor(
                                out=vn_sb[:, :, m, 0:64],
                                in0=vn_sb[:, :, m, 0:64],
                                in1=rk_sb[:, :, m].unsqueeze(-1)
                                .to_broadcast([128, 16, 64]),
                                op=ALU.mult)
                            nc.gpsimd.tensor_copy(
                                out=vn_sb[:, :, m, 64:65],
                                in_=rk_sb[:, :, m].unsqueeze(-1))

                def scan(m, qp, kp, kt):
                    p = m % 2
                    st = stp.tile([128, 4, 66], bf16, tag=f"st{p}",
                                  name=f"st{p}")
                    nc.vector.memset(st, 0.0)
                    nc.vector.memset(st[:, :, 64:65], EPS_ATTN)
                    stT_ps = stps.tile([66, 512], f32, tag=f"sT{p}",
                                       name=f"sT{p}")
                    stT_sb = stp.tile([66, 512], bf16, tag=f"sTs{p}",
                                      name=f"sTs{p}")
                    for c in range(NCH):
                        tsl = slice(CH * c, CH * (c + 1))
                        o_ps = ops.tile([66, CH], f32, tag="o", name="o")
                        bms = []
                        for jb in range(NJB):
                            jbg = NJB * c + jb
                            cols = CH - 128 * jb
                            jsl = slice(CH * c + 128 * jb, CH * (c + 1))
                            bk = bkps.tile([128, 512], f32, tag="bk",
                                           name="bk")
                            for fc in range(4):
                                nc.tensor.matmul(
                                    bk[:, 0:cols],
                                    kp[:, fc, 128 * jbg:128 * (jbg + 1)],
                                    qp[:, fc, jsl],
                                    start=(fc == 0), stop=(fc == 3))
                            bm = bmp.tile([128, 512], bf16,
                                          tag=f"bm{jb}", name=f"bm{jb}")
                            with nc.allow_low_precision(reason="bf16 B"):
                                nc.vector.tensor_mul(
                                    out=bm[:, 0:cols], in0=bk[:, 0:cols],
                                    in1=mask_sb[:, 0:cols])
                            bms.append(bm)
                        # out^T: intra (starts the psum group) then inter
                        for jb in range(NJB):
                            jbg = NJB * c + jb
                            cols = CH - 128 * jb
                            nc.tensor.matmul(
                                o_ps[:, 128 * jb:CH],
                                vn_sb[:, jbg, m, :], bms[jb][:, 0:cols],
                                start=(jb == 0), stop=False,
                                skip_group_check=True)
                            nc.tensor.matmul(
                                stT_ps[:],
                                vn_sb[:, jbg, m, :], kt[:, jbg, p, :],
                                start=(c == 0 and jb == 0),
                                stop=(c == NCH - 1 and jb == NJB - 1),
                                skip_group_check=True)
                        for fc in range(4):
                            nc.tensor.matmul(
                                o_ps[:], st[:, fc, :], qp[:, fc, tsl],
                                start=False, stop=(fc == 3),
                                skip_group_check=True)
                        if c < NCH - 1:
                            nc.scalar.copy(out=stT_sb[:], in_=stT_ps[:])
                            for fc in range(4):
                                t_ps = trp.tile([128, 66], bf16, tag="tr",
                                                 name="tr")
                                nc.tensor.transpose(
                                    t_ps[:],
                                    stT_sb[:, 128 * fc:128 * (fc + 1)],
                                    ident[0:66, 0:66])
                                nc.scalar.copy(out=st[:, fc, 0:64],
                                               in_=t_ps[:, 0:64])
                                nc.scalar.add(out=st[:, fc, 64:65],
                                              in_=t_ps[:, 64:65],
                                              add=epsc[:, 0:1])
                        nr = rowp.tile([1, CH], f32, tag=f"nr{p}",
                                       name=f"nr{p}")
                        nb = rowp.tile([128, CH], f32, tag=f"nb{p}",
                                       name=f"nb{p}")
                        nc.vector.reciprocal(out=nr[:], in_=o_ps[64:65, :])
                        nc.gpsimd.partition_broadcast(out_ap=nb[:],
                                                      in_ap=nr[:])
                        with nc.allow_low_precision(reason="bf16 out"):
                            nc.vector.tensor_mul(
                                out=onT_sb[64 * p:64 * (p + 1), m // 2,
                                           tsl],
                                in0=o_ps[0:64, :], in1=nb[0:64, :])
                        yield

                def drain(gen):
                    for _ in gen:
                        pass

                # software pipeline: interleave features(m+1) slabs with
                # scan(m) chunks so PE never stalls on activation drains
                feats = {}
                drain(features(0, feats))
                for m in range(1, 4):
                    f_gen = features(m, feats)
                    mm = m - 1
                    s_gen = scan(mm, feats[mm][0], feats[mm][1],
                                 feats[2 * (mm // 2)][2])
                    alive = True
                    while alive:
                        alive = False
                        for _ in range(3):
                            if next(f_gen, 'end') != 'end':
                                alive = True
                        if next(s_gen, 'end') != 'end':
                            alive = True
                    drain(f_gen)
                    drain(s_gen)
                drain(scan(3, feats[3][0], feats[3][1], feats[2][2]))

            # ---- phase 5: partial attn = out_norm @ woT (fp16) ----
            with tc.tile_pool(name="atsb", bufs=2) as atsbp, \
                 tc.tile_pool(name="atps", bufs=2, space="PSUM") as atps:
                part_v = part_d[:].rearrange("(g p) d -> p g d", p=128)
                for g in range(4):
                    a_sb = atsbp.tile([128, 4, DM], fp16, tag="atsb",
                                      name="atsb")
                    for cc in range(4):
                        tb = 4 * g + cc
                        a_ps = atps.tile([128, DM], f32, tag="atps",
                                         name="atps")
                        for pb in range(2):
                            nc.tensor.matmul(
                                a_ps[:],
                                onT_sb[:, pb, 128 * tb:128 * (tb + 1)],
                                wo_sb[:, pb, :],
                                start=(pb == 0), stop=(pb == 1))
                        nc.scalar.copy(out=a_sb[:, cc, :], in_=a_ps[:])
                    nc.sync.dma_start(out=part_v[:, 4 * g:4 * (g + 1), :],
                                      in_=a_sb[:])

    nc.compile()
    return nc


def _host_prep(h, w_qkv, w_o, proj_matrix):
    """Build per-core packed bf16 input maps."""
    import ml_dtypes
    bf = ml_dtypes.bfloat16
    projs = (proj_matrix * (DH ** -0.25)).astype(np.float32)  # (64, 256)
    proj2 = np.concatenate([projs, projs], axis=0)            # (128, 256)
    bdproj = np.zeros((128, 512), np.float32)
    bdproj[0:64, 0:256] = projs
    bdproj[64:128, 256:512] = projs
    ident = np.eye(128, dtype=np.float32)
    tri = (np.arange(128)[:, None] <= np.arange(128)[None, :]
           ).astype(np.float32)
    mask_const = np.concatenate([tri, np.ones((128, 384), np.float32)],
                                axis=1)
    woT_full = (w_o.T * SCALE).astype(np.float32)  # (512, 512)

    in_maps = []
    for core in range(N_CORES):
        b, hg = core // 2, core % 2
        heads = [HPC * hg + mm for mm in range(HPC)]
        hT = np.ascontiguousarray(h[:, b, :].T)
        wpack = np.zeros((DM, 2048), np.float32)
        for g in range(2):
            for p in range(2):
                hh = heads[2 * g + p]
                blk = w_qkv[192 * hh:192 * (hh + 1)]  # (192, DM) [q,k,v]
                c0 = 256 * g + 64 * p
                wpack[:, c0:c0 + 64] = blk[0:64].T
                wpack[:, c0 + 128:c0 + 192] = blk[64:128].T
                i = 2 * g + p
                wpack[:, 512 + 64 * i:512 + 64 * (i + 1)] = blk[128:192].T
        for i, hh in enumerate(heads):
            wpack[64 * i:64 * (i + 1), 768:1280] = \
                woT_full[64 * hh:64 * (hh + 1), :]
        wpack[0:128, 1536:2048] = bdproj
        wpack[128:256, 1536:1792] = proj2
        wpack[128:256, 1792:1920] = ident
        wpack[256:384, 1536:2048] = mask_const
        in_maps.append({
            "hTb": hT.astype(bf),
            "wpack": wpack.astype(bf),
        })
    return in_maps


def kernel(h, w_qkv, w_o, ln_gamma, ln_beta, proj_matrix):
    from concourse.bass_utils import run_bass_kernel_spmd

    h = np.asarray(h, np.float32)
    w_qkv = np.asarray(w_qkv, np.float32)
    w_o = np.asarray(w_o, np.float32)
    ln_gamma = np.asarray(ln_gamma, np.float32)
    ln_beta = np.asarray(ln_beta, np.float32)
    proj_matrix = np.asarray(proj_matrix, np.float32)

    if "nc" not in _CACHE:
        _CACHE["nc"] = _build_nc()
    nc = _CACHE["nc"]

    in_maps = _host_prep(h, w_qkv, w_o, proj_matrix)
    res = run_bass_kernel_spmd(nc, in_maps, core_ids=list(range(N_CORES)))

    out = np.empty((L, B, DM), np.float32)
    for b in range(B):
        attn = (np.asarray(res.results[2 * b]["part"], np.float32)
                + np.asarray(res.results[2 * b + 1]["part"], np.float32))
        x = h[:, b, :] + attn
        mu = x.mean(-1, keepdims=True)
        var = ((x - mu) ** 2).mean(-1, keepdims=True)
        out[:, b, :] = (x - mu) / np.sqrt(var + EPS_LN) * ln_gamma + ln_beta
    return out
